# revision 19
# baseline (speedup 1.0000x reference)
"""Bahdanau-style additive attention on 8 TRN2 NeuronCores.

  hidden = tanh(q @ Wq + k @ Wk)        (B, L, H)
  scores = hidden @ v_param             (B, L)
  attn   = softmax(scores, axis=-1)
  out    = attn @ v                     (B, D)

Sharding: data-parallel over batch — 4 batches per core (B=32, 8 cores).

DMA strategy: SBUF (26MB usable) holds the whole per-core problem (17MB), so
every k and v transfer is issued up front into its own buffer — both queues
always run with deep backlogs.  Both streams are laid out HOST-SIDE as
CONTIGUOUS 512KB blocks per tile: sequential HBM reads run ~400 B/ns per
queue vs ~280 B/ns for partition-strided reads.

Per-core pipeline, software-pipelined in PAIR slots (1024 positions):

  W1  pre[H, 1024] = wk16.T @ k16        2 fp16 matmuls (one per psum bank)
  ACT hh = tanh(pre + qWq_b) -> fp16     1024-wide, per-partition bias
  W2  scol[:, j]   = hh_j.T @ vp16       8 score-column matmuls (lags W1 by 1)
  ACT w = exp(scol) -> bf16, accum_out   one exp per batch (per PAIR for the
                                         last batch, to shorten the tail)
  W3  acc[D, 1]   += v_j.T @ w_col       64-matmul burst 2 slots after exp;
                                         v STATIONARY (weight-load streams 4
                                         cols/cy -> 27ns per 128 positions)
      den[1, ...]  = ones.T @ wsum       cross-partition sum of exp row-sums
  host: out = acc / den

Numerics (validated on host against the f64 reference for these inputs):
k=f16, wk=f16, hidden=f16, vp=f16, w=bf16, v=bf16 -> ~2.6e-3 max rel err
(vs the 2e-2 gate). fp16's 11-bit mantissa keeps score error ~1e-3; bf16 for
w is required for range (w = exp(score), scores up to ~40, no max-subtract).
"""

import ml_dtypes
import numpy as np

import concourse.bass as bass
import concourse.mybir as mybir
from concourse.tile import TileContext

B, L, D, H = 32, 8192, 128, 128
NCORES = 8
BPC = B // NCORES  # batches per core
CHUNK = 512  # L positions per W1 matmul (psum bank limit)
PAIR = 2 * CHUNK  # positions per pipeline slot
NP_B = L // PAIR  # pair slots per batch (8)
NSLOT = BPC * NP_B  # total slots (32)
KTILE = 4096  # L positions per k DMA tile (contiguous 1MB block, 8KB rows)
NKT = L // KTILE  # k tiles per batch (2)
NGT = NKT * BPC  # global k tiles (16)
SUB = 128  # L positions per W2/W3 sub-chunk (stationary width)
VT_COLS = 16  # W3 sub-chunks per v DMA subtile
NVT = L // (SUB * VT_COLS)  # v subtiles per batch (4)
ODV = 1 + L // SUB  # out cols per batch: acc col + 64 partial denominators

F32 = mybir.dt.float32
F16 = mybir.dt.float16
BF16 = mybir.dt.bfloat16
ACTF = mybir.ActivationFunctionType

NFILL = 12  # p-state filler matmuls per slot

_CACHE = {}


def _split_excess_waits(nc, max_waits=1):
    """walrus in this env accepts at most one sync-wait per instruction;
    move extras onto InstNoOps placed just before (same engine, in order)."""
    for fn in nc.m.functions:
        for bb in fn.blocks:
            insts = list(bb.instructions)
            new_insts = []
            for ins in insts:
                si = ins.sync_info
                waits = list(si.on_wait) if si and si.on_wait else []
                if len(waits) > max_waits:
                    extra, keep = waits[:-max_waits], waits[-max_waits:]
                    for g0 in range(0, len(extra), max_waits):
                        pre = mybir.InstNoOp(
                            name=f"{ins.name}-waitsplit{g0}",
                            engine=ins.engine,
                            ins=[],
                            outs=[],
                            sync_info=mybir.SyncInfo(
                                on_wait=extra[g0 : g0 + max_waits], on_update=[]
                            ),
                        )
                        nc.register_instruction(pre, overwrite=True)
                        new_insts.append(pre)
                    ins.sync_info = mybir.SyncInfo(
                        on_wait=keep, on_update=list(si.on_update or [])
                    )
                new_insts.append(ins)
            if len(new_insts) != len(insts):
                bb.instructions[:] = new_insts


def build_nc():
    nc = bass.Bass("TRN2")

    # k: tile-major so every 512KB tile is one contiguous HBM block
    k_in = nc.dram_tensor("k16", [BPC, NKT, D, KTILE], F16, kind="ExternalInput")
    v_in = nc.dram_tensor("vv", [BPC, NVT, SUB, VT_COLS * D], BF16, kind="ExternalInput")
    # packed consts: cols 0:4 qwq (f32), 4:68 wk16 (fp16 pairs), 68 vp16|pad
    cst_in = nc.dram_tensor("cst", [128, 69], F32, kind="ExternalInput")
    out_d = nc.dram_tensor("out", [128, BPC * ODV], F32, kind="ExternalOutput")

    with TileContext(nc) as tc:
        with (
            tc.tile_pool(name="const", bufs=1) as cpool,
            tc.tile_pool(name="kp", bufs=NGT - 1) as kpool,
            tc.tile_pool(name="kcp", bufs=4) as kcpool,
            tc.tile_pool(name="vp_", bufs=BPC) as vpool,
            tc.tile_pool(name="hp", bufs=3) as hpool,
            tc.tile_pool(name="wp", bufs=2) as wpool,
            tc.tile_pool(name="ob", bufs=1) as opool,
            tc.tile_pool(name="pre", bufs=2, space="PSUM") as pre_pool,
            tc.tile_pool(name="sps", bufs=2, space="PSUM") as s_pool,
            tc.tile_pool(name="ops", bufs=2, space="PSUM") as o_pool,
        ):
            # HAM warm-up on zeroed tiles: needs no DMA, so the PE clock
            # gate lifts during the Tile preamble / first k transfer.
            zwarm = cpool.tile([128, 256], BF16)
            nc.gpsimd.memset(zwarm[:], 0.0)
            warm_ps = pre_pool.tile([H, PAIR], F32, tag="pre")
            for _ in range(8):
                nc.tensor.matmul(
                    warm_ps[:, :256], zwarm[:, :128], zwarm[:], start=True, stop=True
                )
            # dummy activation: pulls the ACT function table load (~1.3us)
            # off the critical path, concurrent with the first k transfer
            tdum = cpool.tile([128, 1], F32)
            nc.scalar.activation(tdum[:], zwarm[:, 0:1], ACTF.Tanh)

            cst = cpool.tile([128, 69], F32)
            nc.sync.dma_start(cst[:], cst_in[:])
            qwq = cst[:, 0:4]
            wk = cst[:, 4:68].bitcast(F16)
            vp = cst[:, 68:69].bitcast(F16)[:, 0:1]
            ones = cpool.tile([128, 1], BF16)
            nc.gpsimd.memset(ones[:], 1.0)

            out_sb = opool.tile([128, BPC * ODV], F32)

            # ALL DMA up front on the single SWDGE (gpsimd) queue, in exact
            # consumption order: the HWDGE (sync) queue tops out ~283 B/ns
            # and two concurrent queues arbitrate down to ~340 total, while
            # a deep-backlogged SWDGE queue alone sustains ~390 B/ns.
            kmap = {}
            v_bufs = {}

            def issue_k(g):
                b, i = divmod(g, NKT)
                if g == 0:  # slot-granular 256KB pieces: fast start even
                    # while the per-dma_start issue rate (~650ns) binds
                    for s in range(4):
                        kc = kcpool.tile([D, PAIR], F16, tag="kc", name="kc")
                        nc.gpsimd.dma_start(
                            kc[:], k_in[b, i][:, s * PAIR : (s + 1) * PAIR]
                        )
                        kmap[8 * g + 2 * s] = (kc, 0)
                        kmap[8 * g + 2 * s + 1] = (kc, CHUNK)
                else:
                    kt = kpool.tile([D, KTILE], F16, tag="kt", name="kt")
                    nc.gpsimd.dma_start(kt[:], k_in[b, i])
                    for s in range(8):
                        kmap[8 * g + s] = (kt, s * CHUNK)

            def issue_v(b, vt):
                if vt == 0:
                    v_bufs[b] = vpool.tile(
                        [SUB, NVT * VT_COLS * D], BF16, tag="vb", name="vb"
                    )
                nc.gpsimd.dma_start(
                    v_bufs[b][:, vt * VT_COLS * D : (vt + 1) * VT_COLS * D],
                    v_in[b, vt],
                )

            # need-times: k tile g feeds W1 at slot 2g; v subtile (b, vt)
            # feeds the W3 burst at slot 8b+10+vt (k first on ties)
            units = sorted(
                [(4 * g, 0, ("k", g)) for g in range(NGT)]
                + [(8 * b + 10 + vt, 1, ("v", b, vt)) for b in range(BPC) for vt in range(NVT)]
            )
            for _, _, u in units:
                issue_k(u[1]) if u[0] == "k" else issue_v(u[1], u[2])

            scols, ws = {}, {}
            hhs = {}
            LASTB = BPC - 1

            def w2_block(P):
                b, p = divmod(P, NP_B)
                if p == 0:
                    # cols 64:128 are a throwaway target for p-state filler
                    # matmuls (scores live in cols 0:64)
                    scols[b] = s_pool.tile(
                        [SUB, 2 * (L // SUB)], F32, tag="scol", name="scol"
                    )
                hh = hhs.pop(P)
                for j in range(PAIR // SUB):
                    c = p * (PAIR // SUB) + j
                    nc.tensor.matmul(
                        scols[b][:, c : c + 1],
                        hh[:, j * SUB : (j + 1) * SUB],
                        vp[:],
                        start=True,
                        stop=True,
                    )

            def alloc_w(b):
                ws[b] = wpool.tile([SUB, L // SUB], BF16, tag="w", name="w")

            def exp_batch(b):
                alloc_w(b)
                nc.scalar.activation(ws[b][:], scols[b][:, 0 : L // SUB], ACTF.Exp)

            def exp_pair(b, p):
                if p == 0:
                    alloc_w(b)
                nc.scalar.activation(
                    ws[b][:, p * 8 : (p + 1) * 8],
                    scols[b][:, p * 8 : (p + 1) * 8],
                    ACTF.Exp,
                )

            accs = {}

            def w3_sub(b, cs):
                """W3 matmuls for sub-chunk range cs of batch b (one psum
                accumulation group spanning the whole batch)."""
                if cs.start == 0:
                    accs[b] = o_pool.tile([128, ODV], F32, tag="acc", name="acc")
                for c in range(cs.start, cs.stop):
                    nc.tensor.matmul(
                        accs[b][:, 0:1],
                        v_bufs[b][:, c * D : (c + 1) * D],
                        ws[b][:, c : c + 1],
                        start=(c == 0),
                        stop=(c == L // SUB - 1),
                    )

            def finish_batch(b, nden):
                # 64 partial denominators in one bf16 matmul; the host sums
                # them (keeps exp free of accum_out and the ACT free of the
                # accumulator-read that used to sit on the slot critical path)
                nc.tensor.matmul(
                    accs[b][0:1, 1:ODV],
                    ones[:],
                    ws[b][:],
                    start=True,
                    stop=True,
                )
                nc.vector.tensor_copy(out_sb[:, b * ODV : (b + 1) * ODV], accs[b][:])
                nc.gpsimd.dma_start(
                    out_d[:, b * ODV : (b + 1) * ODV],
                    out_sb[:, b * ODV : (b + 1) * ODV],
                )

            for P in range(NSLOT):
                b, p = divmod(P, NP_B)

                # W1 for this pair: two 512-wide matmuls into one psum tile
                pre = pre_pool.tile([H, PAIR], F32, tag="pre")
                for h in range(2):
                    kt, off = kmap[2 * P + h]
                    nc.tensor.matmul(
                        pre[:, h * CHUNK : (h + 1) * CHUNK],
                        wk[:],
                        kt[:, off : off + CHUNK],
                        start=True,
                        stop=True,
                    )
                hh = hpool.tile([H, PAIR], F16, tag="hh", name="hh")
                nc.scalar.activation(
                    hh[:], pre[:], ACTF.Tanh, bias=qwq[:, b : b + 1], scale=1.0
                )
                hhs[P] = hh

                if P >= 1:
                    sc_f = scols.get((P - 1) // NP_B)
                    if sc_f is not None:
                        for _ in range(NFILL):
                            nc.tensor.matmul(
                                sc_f[:, 64:128], zwarm[:, :128], zwarm[:, :64],
                                start=True, stop=True,
                            )
                    w2_block(P - 1)
                    bb, pp = divmod(P - 1, NP_B)
                    if bb == LASTB:
                        exp_pair(bb, pp)  # per-pair on the last batch: short tail
                elif P == 0:
                    pass
                if P % NP_B == 0 and P > 0 and P // NP_B - 1 != LASTB:
                    exp_batch(P // NP_B - 1)
                if P % NP_B in (2, 3, 4, 5) and P > NP_B and P // NP_B - 1 != LASTB:
                    bw = P // NP_B - 1
                    vt = P % NP_B - 2
                    w3_sub(bw, slice(vt * VT_COLS, (vt + 1) * VT_COLS))
                    if vt == NVT - 1:
                        finish_batch(bw, 1)
                # last batch: W3 per pair, trailing its exp by one slot
                bb, pp = divmod(P - 2, NP_B)
                if bb == LASTB and pp >= 0:
                    w3_sub(LASTB, slice(pp * 8, (pp + 1) * 8))

            w2_block(NSLOT - 1)
            exp_pair(LASTB, NP_B - 1)
            w3_sub(LASTB, slice((NP_B - 2) * 8, (NP_B - 1) * 8))
            w3_sub(LASTB, slice((NP_B - 1) * 8, NP_B * 8))
            finish_batch(LASTB, NP_B)

    _split_excess_waits(nc)
    return nc


def _prep_inputs(q, k, v, W_line, v_param):
    """Host-side shard + layout prep. Returns per-core input maps."""
    qWq = q.astype(np.float64) @ W_line[:D].astype(np.float64)  # (B, H)
    wk16 = np.ascontiguousarray(W_line[D:]).astype(np.float16)  # (D, H)
    vp16 = np.zeros((H, 2), dtype=np.float16)
    vp16[:, 0] = v_param.astype(np.float16)

    cst_base = np.zeros((128, 69), dtype=np.float32)
    cst_base[:, 4:68] = wk16.view(np.float32)
    cst_base[:, 68:69] = vp16.view(np.float32)

    in_maps = []
    for c in range(NCORES):
        bs = slice(c * BPC, (c + 1) * BPC)
        # k tile-major: [b][tile][d][col], each tile one contiguous 512KB block
        k16 = np.ascontiguousarray(
            k[bs]
            .transpose(0, 2, 1)
            .reshape(BPC, D, NKT, KTILE)
            .transpose(0, 2, 1, 3)
        ).astype(np.float16)
        # v into the SBUF tile layout: [b][vt][p][col*D+d], bf16
        vv = np.ascontiguousarray(
            v[bs]
            .reshape(BPC, NVT, VT_COLS, SUB, D)
            .transpose(0, 1, 3, 2, 4)
            .reshape(BPC, NVT, SUB, VT_COLS * D)
        ).astype(ml_dtypes.bfloat16)
        cst = cst_base.copy()
        cst[:, 0:4] = qWq[bs].T.astype(np.float32)
        in_maps.append({"k16": k16, "vv": vv, "cst": cst})
    return in_maps


def _gather_output(results):
    out = np.empty((B, D), dtype=np.float32)
    for c, r in enumerate(results):
        cols = r["out"].astype(np.float64)  # [128, BPC*ODV]
        for b in range(BPC):
            den = cols[0, b * ODV + 1 : (b + 1) * ODV].sum()
            out[c * BPC + b] = (cols[:, b * ODV] / den).astype(np.float32)
    return out


def run(q, k, v, W_line, v_param, trace=False, **spmd_kwargs):
    from concourse.bass_utils import run_bass_kernel_spmd

    if "nc" not in _CACHE:
        _CACHE["nc"] = build_nc()
    nc = _CACHE["nc"]
    in_maps = _prep_inputs(q, k, v, W_line, v_param)
    res = run_bass_kernel_spmd(
        nc, in_maps, list(range(NCORES)), trace=trace, **spmd_kwargs
    )
    return _gather_output(res.results), res


def kernel(q, k, v, W_line, v_param):
    out, _ = run(q, k, v, W_line, v_param, trace=False)
    return out


# revision 20
# speedup vs baseline: 1.0504x; 1.0504x over previous
"""Bahdanau-style additive attention on 8 TRN2 NeuronCores.

  hidden = tanh(q @ Wq + k @ Wk)        (B, L, H)
  scores = hidden @ v_param             (B, L)
  attn   = softmax(scores, axis=-1)
  out    = attn @ v                     (B, D)

Sharding: data-parallel over batch — 4 batches per core (B=32, 8 cores).

DMA strategy: SBUF (26MB usable) holds the whole per-core problem (17MB), so
every k and v transfer is issued up front into its own buffer — both queues
always run with deep backlogs.  Both streams are laid out HOST-SIDE as
CONTIGUOUS 512KB blocks per tile: sequential HBM reads run ~400 B/ns per
queue vs ~280 B/ns for partition-strided reads.

Per-core pipeline, software-pipelined in PAIR slots (1024 positions):

  W1  pre[H, 1024] = wk16.T @ k16        2 fp16 matmuls (one per psum bank)
  ACT hh = tanh(pre + qWq_b) -> fp16     1024-wide, per-partition bias
  W2  scol[:, j]   = hh_j.T @ vp16       8 score-column matmuls (lags W1 by 1)
  ACT w = exp(scol) -> bf16, accum_out   one exp per batch (per PAIR for the
                                         last batch, to shorten the tail)
  W3  acc[D, 1]   += v_j.T @ w_col       64-matmul burst 2 slots after exp;
                                         v STATIONARY (weight-load streams 4
                                         cols/cy -> 27ns per 128 positions)
      den[1, ...]  = ones.T @ wsum       cross-partition sum of exp row-sums
  host: out = acc / den

Numerics (validated on host against the f64 reference for these inputs):
k=f16, wk=f16, hidden=f16, vp=f16, w=bf16, v=bf16 -> ~2.6e-3 max rel err
(vs the 2e-2 gate). fp16's 11-bit mantissa keeps score error ~1e-3; bf16 for
w is required for range (w = exp(score), scores up to ~40, no max-subtract).
"""

import ml_dtypes
import numpy as np

import concourse.bass as bass
import concourse.mybir as mybir
from concourse.tile import TileContext

B, L, D, H = 32, 8192, 128, 128
NCORES = 8
BPC = B // NCORES  # batches per core
CHUNK = 512  # L positions per W1 matmul (psum bank limit)
PAIR = 2 * CHUNK  # positions per pipeline slot
NP_B = L // PAIR  # pair slots per batch (8)
NSLOT = BPC * NP_B  # total slots (32)
KTILE = 4096  # L positions per k DMA tile (contiguous 1MB block, 8KB rows)
NKT = L // KTILE  # k tiles per batch (2)
NGT = NKT * BPC  # global k tiles (16)
SUB = 128  # L positions per W2/W3 sub-chunk (stationary width)
VT_COLS = 16  # W3 sub-chunks per v DMA subtile
NVT = L // (SUB * VT_COLS)  # v subtiles per batch (4)
ODV = 1 + L // SUB  # out cols per batch: acc col + 64 partial denominators

F32 = mybir.dt.float32
F16 = mybir.dt.float16
BF16 = mybir.dt.bfloat16
ACTF = mybir.ActivationFunctionType

NFILL = 12  # p-state filler matmuls per slot

_CACHE = {}


def _split_excess_waits(nc, max_waits=1):
    """walrus in this env accepts at most one sync-wait per instruction;
    move extras onto InstNoOps placed just before (same engine, in order)."""
    for fn in nc.m.functions:
        for bb in fn.blocks:
            insts = list(bb.instructions)
            new_insts = []
            for ins in insts:
                si = ins.sync_info
                waits = list(si.on_wait) if si and si.on_wait else []
                if len(waits) > max_waits:
                    extra, keep = waits[:-max_waits], waits[-max_waits:]
                    for g0 in range(0, len(extra), max_waits):
                        pre = mybir.InstNoOp(
                            name=f"{ins.name}-waitsplit{g0}",
                            engine=ins.engine,
                            ins=[],
                            outs=[],
                            sync_info=mybir.SyncInfo(
                                on_wait=extra[g0 : g0 + max_waits], on_update=[]
                            ),
                        )
                        nc.register_instruction(pre, overwrite=True)
                        new_insts.append(pre)
                    ins.sync_info = mybir.SyncInfo(
                        on_wait=keep, on_update=list(si.on_update or [])
                    )
                new_insts.append(ins)
            if len(new_insts) != len(insts):
                bb.instructions[:] = new_insts


def build_nc():
    nc = bass.Bass("TRN2")

    # k: tile-major so every 512KB tile is one contiguous HBM block
    k_in = nc.dram_tensor("k16", [BPC, NKT, D, KTILE], F16, kind="ExternalInput")
    v_in = nc.dram_tensor("vv", [BPC, NVT, SUB, VT_COLS * D], BF16, kind="ExternalInput")
    # packed consts: cols 0:4 qwq (f32), 4:68 wk16 (fp16 pairs), 68 vp16|pad
    cst_in = nc.dram_tensor("cst", [128, 69], F32, kind="ExternalInput")
    out_d = nc.dram_tensor("out", [128, BPC * ODV], F32, kind="ExternalOutput")

    with TileContext(nc) as tc:
        with (
            tc.tile_pool(name="const", bufs=1) as cpool,
            tc.tile_pool(name="kp", bufs=NGT - 1) as kpool,
            tc.tile_pool(name="kcp", bufs=4) as kcpool,
            tc.tile_pool(name="vp_", bufs=BPC) as vpool,
            tc.tile_pool(name="hp", bufs=3) as hpool,
            tc.tile_pool(name="wp", bufs=2) as wpool,
            tc.tile_pool(name="ob", bufs=1) as opool,
            tc.tile_pool(name="pre", bufs=2, space="PSUM") as pre_pool,
            tc.tile_pool(name="sps", bufs=2, space="PSUM") as s_pool,
            tc.tile_pool(name="ops", bufs=2, space="PSUM") as o_pool,
        ):
            # HAM warm-up on zeroed tiles: needs no DMA, so the PE clock
            # gate lifts during the Tile preamble / first k transfer.
            zwarm = cpool.tile([128, 256], BF16)
            nc.gpsimd.memset(zwarm[:], 0.0)
            warm_ps = pre_pool.tile([H, PAIR], F32, tag="pre")
            for _ in range(8):
                nc.tensor.matmul(
                    warm_ps[:, :256], zwarm[:, :128], zwarm[:], start=True, stop=True
                )
            # dummy activation: pulls the ACT function table load (~1.3us)
            # off the critical path, concurrent with the first k transfer
            tdum = cpool.tile([128, 1], F32)
            nc.scalar.activation(tdum[:], zwarm[:, 0:1], ACTF.Tanh)

            cst = cpool.tile([128, 69], F32)
            nc.sync.dma_start(cst[:], cst_in[:])
            qwq = cst[:, 0:4]
            wk = cst[:, 4:68].bitcast(F16)
            vp = cst[:, 68:69].bitcast(F16)[:, 0:1]
            ones = cpool.tile([128, 1], BF16)
            nc.gpsimd.memset(ones[:], 1.0)

            out_sb = opool.tile([128, BPC * ODV], F32)

            # ALL DMA up front on the single SWDGE (gpsimd) queue, in exact
            # consumption order: the HWDGE (sync) queue tops out ~283 B/ns
            # and two concurrent queues arbitrate down to ~340 total, while
            # a deep-backlogged SWDGE queue alone sustains ~390 B/ns.
            kmap = {}
            v_bufs = {}

            def issue_k(g):
                b, i = divmod(g, NKT)
                if g == 0:  # slot-granular 256KB pieces: fast start even
                    # while the per-dma_start issue rate (~650ns) binds
                    for s in range(4):
                        kc = kcpool.tile([D, PAIR], F16, tag="kc", name="kc")
                        nc.gpsimd.dma_start(
                            kc[:], k_in[b, i][:, s * PAIR : (s + 1) * PAIR]
                        )
                        kmap[8 * g + 2 * s] = (kc, 0)
                        kmap[8 * g + 2 * s + 1] = (kc, CHUNK)
                else:
                    kt = kpool.tile([D, KTILE], F16, tag="kt", name="kt")
                    nc.gpsimd.dma_start(kt[:], k_in[b, i])
                    for s in range(8):
                        kmap[8 * g + s] = (kt, s * CHUNK)

            def issue_v(b, vt):
                if vt == 0:
                    v_bufs[b] = vpool.tile(
                        [SUB, NVT * VT_COLS * D], BF16, tag="vb", name="vb"
                    )
                nc.gpsimd.dma_start(
                    v_bufs[b][:, vt * VT_COLS * D : (vt + 1) * VT_COLS * D],
                    v_in[b, vt],
                )

            # need-times: k tile g feeds W1 at slot 2g; v subtile (b, vt)
            # feeds the W3 burst at slot 8b+10+vt (k first on ties)
            units = sorted(
                [(4 * g, 0, ("k", g)) for g in range(NGT)]
                + [(8 * b + 10 + vt, 1, ("v", b, vt)) for b in range(BPC) for vt in range(NVT)]
            )
            for _, _, u in units:
                issue_k(u[1]) if u[0] == "k" else issue_v(u[1], u[2])

            scols, ws = {}, {}
            hhs = {}
            LASTB = BPC - 1

            def w2_block(P):
                b, p = divmod(P, NP_B)
                if p == 0:
                    # cols 64:128 are a throwaway target for p-state filler
                    # matmuls (scores live in cols 0:64)
                    scols[b] = s_pool.tile(
                        [SUB, 2 * (L // SUB)], F32, tag="scol", name="scol"
                    )
                hh = hhs.pop(P)
                for j in range(PAIR // SUB):
                    c = p * (PAIR // SUB) + j
                    nc.tensor.matmul(
                        scols[b][:, c : c + 1],
                        hh[:, j * SUB : (j + 1) * SUB],
                        vp[:],
                        start=True,
                        stop=True,
                    )

            def alloc_w(b):
                ws[b] = wpool.tile([SUB, L // SUB], BF16, tag="w", name="w")

            def exp_batch(b):
                alloc_w(b)
                nc.scalar.activation(ws[b][:], scols[b][:, 0 : L // SUB], ACTF.Exp)

            def exp_pair(b, p):
                if p == 0:
                    alloc_w(b)
                nc.scalar.activation(
                    ws[b][:, p * 8 : (p + 1) * 8],
                    scols[b][:, p * 8 : (p + 1) * 8],
                    ACTF.Exp,
                )

            accs = {}

            def w3_sub(b, cs):
                """W3 matmuls for sub-chunk range cs of batch b (one psum
                accumulation group spanning the whole batch)."""
                if cs.start == 0:
                    accs[b] = o_pool.tile([128, ODV], F32, tag="acc", name="acc")
                for c in range(cs.start, cs.stop):
                    nc.tensor.matmul(
                        accs[b][:, 0:1],
                        v_bufs[b][:, c * D : (c + 1) * D],
                        ws[b][:, c : c + 1],
                        start=(c == 0),
                        stop=(c == L // SUB - 1),
                    )

            def finish_batch(b, nden):
                # 64 partial denominators in one bf16 matmul; the host sums
                # them (keeps exp free of accum_out and the ACT free of the
                # accumulator-read that used to sit on the slot critical path)
                nc.tensor.matmul(
                    accs[b][0:1, 1:ODV],
                    ones[:],
                    ws[b][:],
                    start=True,
                    stop=True,
                )
                nc.vector.tensor_copy(out_sb[:, b * ODV : (b + 1) * ODV], accs[b][:])
                nc.sync.dma_start(
                    out_d[:, b * ODV : (b + 1) * ODV],
                    out_sb[:, b * ODV : (b + 1) * ODV],
                )

            for P in range(NSLOT):
                b, p = divmod(P, NP_B)

                # W1 for this pair: two 512-wide matmuls into one psum tile
                pre = pre_pool.tile([H, PAIR], F32, tag="pre")
                for h in range(2):
                    kt, off = kmap[2 * P + h]
                    nc.tensor.matmul(
                        pre[:, h * CHUNK : (h + 1) * CHUNK],
                        wk[:],
                        kt[:, off : off + CHUNK],
                        start=True,
                        stop=True,
                    )
                hh = hpool.tile([H, PAIR], F16, tag="hh", name="hh")
                nc.scalar.activation(
                    hh[:], pre[:], ACTF.Tanh, bias=qwq[:, b : b + 1], scale=1.0
                )
                hhs[P] = hh

                if P >= 1:
                    sc_f = scols.get((P - 1) // NP_B)
                    if sc_f is not None:
                        for _ in range(NFILL):
                            nc.tensor.matmul(
                                sc_f[:, 64:128], zwarm[:, :128], zwarm[:, :64],
                                start=True, stop=True,
                            )
                    w2_block(P - 1)
                    bb, pp = divmod(P - 1, NP_B)
                    if bb == LASTB:
                        exp_pair(bb, pp)  # per-pair on the last batch: short tail
                elif P == 0:
                    pass
                if P % NP_B == 0 and P > 0 and P // NP_B - 1 != LASTB:
                    exp_batch(P // NP_B - 1)
                if P % NP_B in (2, 3, 4, 5) and P > NP_B and P // NP_B - 1 != LASTB:
                    bw = P // NP_B - 1
                    vt = P % NP_B - 2
                    w3_sub(bw, slice(vt * VT_COLS, (vt + 1) * VT_COLS))
                    if vt == NVT - 1:
                        finish_batch(bw, 1)
                # last batch: W3 per pair, trailing its exp by one slot
                bb, pp = divmod(P - 2, NP_B)
                if bb == LASTB and pp >= 0:
                    w3_sub(LASTB, slice(pp * 8, (pp + 1) * 8))

            w2_block(NSLOT - 1)
            exp_pair(LASTB, NP_B - 1)
            w3_sub(LASTB, slice((NP_B - 2) * 8, (NP_B - 1) * 8))
            w3_sub(LASTB, slice((NP_B - 1) * 8, NP_B * 8))
            finish_batch(LASTB, NP_B)

    _split_excess_waits(nc)
    return nc


def _prep_inputs(q, k, v, W_line, v_param):
    """Host-side shard + layout prep. Returns per-core input maps."""
    qWq = q.astype(np.float64) @ W_line[:D].astype(np.float64)  # (B, H)
    wk16 = np.ascontiguousarray(W_line[D:]).astype(np.float16)  # (D, H)
    vp16 = np.zeros((H, 2), dtype=np.float16)
    vp16[:, 0] = v_param.astype(np.float16)

    cst_base = np.zeros((128, 69), dtype=np.float32)
    cst_base[:, 4:68] = wk16.view(np.float32)
    cst_base[:, 68:69] = vp16.view(np.float32)

    in_maps = []
    for c in range(NCORES):
        bs = slice(c * BPC, (c + 1) * BPC)
        # k tile-major: [b][tile][d][col], each tile one contiguous 512KB block
        k16 = np.ascontiguousarray(
            k[bs]
            .transpose(0, 2, 1)
            .reshape(BPC, D, NKT, KTILE)
            .transpose(0, 2, 1, 3)
        ).astype(np.float16)
        # v into the SBUF tile layout: [b][vt][p][col*D+d], bf16
        vv = np.ascontiguousarray(
            v[bs]
            .reshape(BPC, NVT, VT_COLS, SUB, D)
            .transpose(0, 1, 3, 2, 4)
            .reshape(BPC, NVT, SUB, VT_COLS * D)
        ).astype(ml_dtypes.bfloat16)
        cst = cst_base.copy()
        cst[:, 0:4] = qWq[bs].T.astype(np.float32)
        in_maps.append({"k16": k16, "vv": vv, "cst": cst})
    return in_maps


def _gather_output(results):
    out = np.empty((B, D), dtype=np.float32)
    for c, r in enumerate(results):
        cols = r["out"].astype(np.float64)  # [128, BPC*ODV]
        for b in range(BPC):
            den = cols[0, b * ODV + 1 : (b + 1) * ODV].sum()
            out[c * BPC + b] = (cols[:, b * ODV] / den).astype(np.float32)
    return out


def run(q, k, v, W_line, v_param, trace=False, **spmd_kwargs):
    from concourse.bass_utils import run_bass_kernel_spmd

    if "nc" not in _CACHE:
        _CACHE["nc"] = build_nc()
    nc = _CACHE["nc"]
    in_maps = _prep_inputs(q, k, v, W_line, v_param)
    res = run_bass_kernel_spmd(
        nc, in_maps, list(range(NCORES)), trace=trace, **spmd_kwargs
    )
    return _gather_output(res.results), res


def kernel(q, k, v, W_line, v_param):
    out, _ = run(q, k, v, W_line, v_param, trace=False)
    return out
